# revision 1
# baseline (speedup 1.0000x reference)
"""Trainium2 Bass kernel for a 2-layer GCN (nn_ClusterGNN):
    h1 = relu(gcn_conv(x, W1, b1)); out = log_softmax(gcn_conv(h1, W2, b2))

Strategy (8 NeuronCores, dest-node sharded):
  - The GCN symmetric norm factorizes: msg(r->c) = dinv[r]*dinv[c]*h[r].
    dinv[src] is folded into x on the host, the dinv[dst] factor is
    deferred into downstream per-partition scales, so edge aggregation
    is a pure unweighted segment-sum.
  - Segment-sum runs on the tensor engine: per 128-edge block, a one-hot
    selector S[e, d] = (col_local[e] == d) is built with one
    tensor_scalar(is_equal) in fp16 (split across DVE and Pool engines),
    then layer 1 accumulates G^T[f, 128] += msg[128, F].T @ S[128, 128]
    and layer 2 accumulates G[128, F] += S[128, 128].T @ msg[128, F]
    (selector as stationary weights; Ldweights is free, so the matmul
    cost is the output free dim: 32 instead of 128).
  - Edge messages come from raw InstDMAGatherAnt (fp16 rows, 128B for
    layer 1 / 64B for layer 2, 256B row stride) out of a per-core
    replicated table; int16 indices are window-local (32768-row windows).
    Gathers are issued one chunk ahead of compute to keep the DMA
    engines busy.
  - Layer-2 table  dinv^2 * (relu(G1) @ W2)  is computed per dest shard
    compactly, AllGathered (6.4MB), and expanded into the strided table.
"""

import os
import sys

if "/opt/trn_rl_repo" not in sys.path:
    sys.path.insert(0, "/opt/trn_rl_repo")

import numpy as np

from concourse import bass, bacc, mybir, tile
from concourse.bass_utils import run_bass_kernel_spmd

P = 128
NCORES = 8
WIN = 32768
CHUNK_T = int(os.environ.get("BG_CHUNK", "14"))  # dest tiles per gather chunk
TROW = 128    # table row stride in fp16 elements (= 256B)
POOL_SEL = int(os.environ.get("BG_POOLSEL", "0"))  # every POOL_SEL-th selector on Pool (0 = all DVE)
MBUF = int(os.environ.get("BG_MBUF", "2"))       # msg tile buffers (chunks in flight)

F32 = mybir.dt.float32
F16 = mybir.dt.float16
I16 = mybir.dt.int16


def cdiv(a, b):
    return -(-a // b)


class Cfg:
    pass


def raw_gather(nc, out_ap, in_ap, idxs_ap, num_idxs, elem_size):
    """InstDMAGatherAnt with arbitrary elem_size (bytes%256 need not hold);
    row stride fixed at 256B (stride_bytes_256=1)."""
    gp = nc.gpsimd
    _in_ap = gp.lower_ap_dma(in_ap, for_custom_bir_dma=True)
    _idxs_ap = gp.lower_ap(idxs_ap)
    _out_ap = gp.lower_ap(out_ap)
    return gp.add_instruction(
        mybir.InstDMAGatherAnt(
            name=nc.get_next_instruction_name(),
            ins=[*_in_ap, _idxs_ap, gp.lower_val_access(gp.to_reg(num_idxs))],
            outs=[_out_ap],
            transpose=False,
            num_idxs=num_idxs,
            elem_size=elem_size,
            stride_bytes_256=1,
            gen_mode=0,
            single_packet=False,
            queue_num=0,
            sbuf_tokens_per_rank=0,
            sbuf_free_dim_per_rank=0,
            sbuf_free_dim_pad_per_rank=0,
            sbuf_byte_offset=0,
        )
    )


def prep(x, edge_index, W1, b1, W2, b2):
    """Host-side layout prep (shard, sort, window-group, pad)."""
    x = np.asarray(x, dtype=np.float32)
    W1 = np.asarray(W1, dtype=np.float32)
    b1 = np.asarray(b1, dtype=np.float32)
    W2 = np.asarray(W2, dtype=np.float32)
    b2 = np.asarray(b2, dtype=np.float32)

    N, FIN = x.shape
    FH = W1.shape[1]
    FO = W2.shape[1]

    cfg = Cfg()
    cfg.N, cfg.FIN, cfg.FH, cfg.FO = N, FIN, FH, FO
    SHARD_T = cdiv(N, P * NCORES)
    NT = SHARD_T * NCORES
    PAD_N = NT * P
    SHARD_N = SHARD_T * P
    cfg.SHARD_T, cfg.NT, cfg.PAD_N, cfg.SHARD_N = SHARD_T, NT, PAD_N, SHARD_N
    NW = cdiv(PAD_N, WIN)
    cfg.NW = NW
    cfg.win_rows = [min(PAD_N, (w + 1) * WIN) - w * WIN for w in range(NW)]
    cfg.use_b1 = bool(np.any(b1))
    cfg.use_b2 = bool(np.any(b2))

    row = np.asarray(edge_index[0], dtype=np.int64)
    col = np.asarray(edge_index[1], dtype=np.int64)
    loops = np.arange(N, dtype=np.int64)
    src_all = np.concatenate([row, loops])
    dst_all = np.concatenate([col, loops])

    # sort edges by (dest tile, window, dst)
    tile_of = dst_all // P
    win_of = src_all // WIN
    key = (tile_of * NW + win_of) * np.int64(PAD_N) + dst_all
    order = np.argsort(key, kind="stable")
    src_s = src_all[order]
    dst_s = dst_all[order]
    tw_s = (tile_of * NW + win_of)[order]

    # counts per (global tile, window) -> shared nblk (max over cores)
    cnts = np.bincount(tw_s, minlength=NT * NW).reshape(NCORES, SHARD_T, NW)
    nblk = cdiv(cnts, P).max(axis=0)  # [SHARD_T, NW]
    cfg.nblk = nblk

    # chunks of dest tiles
    chunks = []
    j = 0
    while j < SHARD_T:
        chunks.append((j, min(j + CHUNK_T, SHARD_T)))
        j += CHUNK_T
    cfg.chunks = chunks

    # slot layout (shared): for chunk: for w: for j in chunk
    slot_start = np.zeros((SHARD_T, NW), dtype=np.int64)
    gathers = []  # per chunk: (w, slot0, nslots)
    blocks_of_tile = [[] for _ in range(SHARD_T)]
    chunk_slot0 = []
    s = 0
    for (j0, j1) in chunks:
        chunk_slot0.append(s)
        glist = []
        for w in range(NW):
            sw0 = s
            for j in range(j0, j1):
                slot_start[j, w] = s
                nb = int(nblk[j, w])
                blocks_of_tile[j].extend(range(s // P, s // P + nb))
                s += nb * P
            if s > sw0:
                glist.append((w, sw0, s - sw0))
        gathers.append(glist)
    SLOTS = s
    NB = SLOTS // P
    cfg.SLOTS, cfg.NB = SLOTS, NB
    cfg.gathers = gathers
    cfg.chunk_slot0 = chunk_slot0
    cfg.blocks_of_tile = blocks_of_tile

    # degree (incl self-loop) -> dinv, computed on host
    deg_edges = np.bincount(col, minlength=PAD_N).astype(np.float64)
    deg = deg_edges + 1.0
    deg[N:] = 1.0
    dinv = (deg ** -0.5).astype(np.float32)
    dinv[N:] = 0.0

    # x^T fp16 pre-scaled by dinv[src], padded (plain node order; the
    # table write uses a (m p)-order 3D access pattern, no permute needed)
    xT_perm = np.zeros((FIN, PAD_N), dtype=np.float16)
    xT_perm[:, :N] = (x * dinv[:N, None]).T.astype(np.float16)

    iota = np.broadcast_to(np.arange(P, dtype=np.float16)[None, :], (P, P)).copy()
    W1h = W1.astype(np.float16)          # [FIN, FH] unpadded
    W2h = W2.astype(np.float16)          # [FH, FO]
    b2rep = np.broadcast_to(b2[None, :], (P, FO)).copy()
    b1r = b1[None, :].copy()

    rp_tiles = np.searchsorted(tw_s, np.arange(NT * NW + 1))

    in_maps = []
    for k in range(NCORES):
        idx_flat = np.zeros(SLOTS, dtype=np.int16)
        col_flat = np.full(SLOTS, -1.0, dtype=np.float32)
        for j in range(SHARD_T):
            t = k * SHARD_T + j
            for w in range(NW):
                a, b = rp_tiles[t * NW + w], rp_tiles[t * NW + w + 1]
                n = b - a
                if n == 0:
                    continue
                s0 = slot_start[j, w]
                idx_flat[s0:s0 + n] = (src_s[a:b] - w * WIN).astype(np.int16)
                col_flat[s0:s0 + n] = (dst_s[a:b] - t * P).astype(np.float32)
        idx16 = np.tile(idx_flat.reshape(-1, 16).T, (NCORES, 1))
        col_arr = col_flat.reshape(NB, P).T.copy()

        gn = (k * SHARD_N + np.arange(SHARD_N)).reshape(SHARD_T, P).T
        dg = dinv[gn]
        in_map = {
            "xT": xT_perm,
            "w1h": W1h,
            "w2h": W2h,
            "iota": iota,
            "idx16": idx16,
            "colv": col_arr,
            "dinv_g": dg.copy(),
            "dinv_gsq": (dg * dg).copy(),
        }
        if cfg.use_b2:
            in_map["b2rep"] = b2rep
        if cfg.use_b1:
            rn = k * SHARD_N + np.arange(SHARD_N)
            in_map["b1r"] = b1r
            # sqrt(deg) = 1/dinv (safe: pad nodes never relu'd into output)
            sq = np.where(dinv[rn] > 0, 1.0 / np.maximum(dinv[rn], 1e-30), 0.0)
            in_map["sqd_r"] = sq[None, :].astype(np.float32).copy()
        in_maps.append(in_map)

    return cfg, in_maps


def build_program(cfg):
    FIN, FH, FO = cfg.FIN, cfg.FH, cfg.FO
    NT, SHARD_T, PAD_N, SHARD_N = cfg.NT, cfg.SHARD_T, cfg.PAD_N, cfg.SHARD_N
    NW, NB, SLOTS = cfg.NW, cfg.NB, cfg.SLOTS

    BUF = int(os.environ.get("BG_BUF", "0"))
    nc = bacc.Bacc(
        "TRN2", target_bir_lowering=False, debug=False, num_devices=NCORES
    )

    xT_in = nc.dram_tensor("xT", [FIN, PAD_N], F16, kind="ExternalInput").ap()
    w1h_in = nc.dram_tensor("w1h", [FIN, FH], F16, kind="ExternalInput").ap()
    w2h_in = nc.dram_tensor("w2h", [FH, FO], F16, kind="ExternalInput").ap()
    iota_in = nc.dram_tensor("iota", [P, P], F16, kind="ExternalInput").ap()
    idx_in = nc.dram_tensor("idx16", [P, SLOTS // 16], I16, kind="ExternalInput").ap()
    col_in = nc.dram_tensor("colv", [P, NB], F32, kind="ExternalInput").ap()
    dinv_g_in = nc.dram_tensor("dinv_g", [P, SHARD_T], F32, kind="ExternalInput").ap()
    dinv_gsq_in = nc.dram_tensor(
        "dinv_gsq", [P, SHARD_T], F32, kind="ExternalInput"
    ).ap()
    if cfg.use_b2:
        b2rep_in = nc.dram_tensor("b2rep", [P, FO], F32, kind="ExternalInput").ap()
    if cfg.use_b1:
        b1r_in = nc.dram_tensor("b1r", [1, FH], F32, kind="ExternalInput").ap()
        sqd_in = nc.dram_tensor("sqd_r", [1, SHARD_N], F32, kind="ExternalInput").ap()

    table1 = nc.dram_tensor("table1", [PAD_N, TROW], F16, kind="Internal").ap()
    t2c = nc.dram_tensor("t2c", [SHARD_N, FO], F16, kind="Internal").ap()
    t2full = nc.dram_tensor(
        "t2full", [PAD_N, FO], F16, kind="Internal", addr_space="Shared"
    ).ap()
    table2 = nc.dram_tensor("table2", [PAD_N, TROW], F16, kind="Internal").ap()
    out = nc.dram_tensor("out", [SHARD_N, FO], F32, kind="ExternalOutput").ap()

    stage = os.environ.get("BASSGNN_STAGE", "full")
    sel_cnt = [0]

    with tile.TileContext(nc) as tc:
        with tc.tile_pool(name="const", bufs=1) as cpool:
            w1_t = cpool.tile([FIN, FH], F16)
            nc.sync.dma_start(out=w1_t[:], in_=w1h_in[:, :])
            w2_t = cpool.tile([FH, FO], F16)
            nc.sync.dma_start(out=w2_t[:], in_=w2h_in[:, :])
            iota_t = cpool.tile([P, P], F16)
            nc.sync.dma_start(out=iota_t[:], in_=iota_in[:, :])
            col_t = cpool.tile([P, NB], F32)
            nc.sync.dma_start(out=col_t[:], in_=col_in[:, :])
            idx_t = cpool.tile([P, SLOTS // 16], I16)
            nc.sync.dma_start(out=idx_t[:], in_=idx_in[:, :])
            dinv_g = cpool.tile([P, SHARD_T], F32)
            nc.sync.dma_start(out=dinv_g[:], in_=dinv_g_in[:, :])
            dinv_gsq = cpool.tile([P, SHARD_T], F32)
            nc.sync.dma_start(out=dinv_gsq[:], in_=dinv_gsq_in[:, :])
            if cfg.use_b2:
                b2_t = cpool.tile([P, FO], F32)
                nc.sync.dma_start(out=b2_t[:], in_=b2rep_in[:, :])
            if cfg.use_b1:
                b1_t = cpool.tile([1, FH], F32)
                nc.sync.dma_start(out=b1_t[:], in_=b1r_in[:, :])
                sqd_t = cpool.tile([1, SHARD_N], F32)
                nc.sync.dma_start(out=sqd_t[:], in_=sqd_in[:, :])

            # ---- Phase B: table1 = (dinv*x) @ W1, full table per core ----
            # 16 node-tiles per group; 8 matmul outputs packed per PSUM bank
            # ([P, 8*FH] fp32 = 2KB); one fp32->fp16 copy per bank,
            # alternating Activation / DVE.
            with (
                tc.tile_pool(name="phb", bufs=3 + BUF) as bpool,
                tc.tile_pool(name="phb_st", bufs=3 + BUF) as stpool,
                tc.tile_pool(name="phb_ps", bufs=4, space="PSUM") as bpsum,
            ):
                GT = 16
                assert NT % GT == 0
                BPB = 512 // FH   # matmul tiles per PSUM bank
                for g8 in range(NT // GT):
                    xt8 = bpool.tile([P, GT * P], F16, tag="xt")
                    nc.sync.dma_start(
                        out=xt8[:], in_=xT_in[:, g8 * GT * P:(g8 + 1) * GT * P]
                    )
                    st8 = stpool.tile([P, GT * FH], F16, tag="st")
                    for bk in range(GT // BPB):
                        hp = bpsum.tile([P, BPB * FH], F32, tag="hp")
                        for i in range(BPB):
                            t = bk * BPB + i
                            nc.tensor.matmul(
                                out=hp[:, i * FH:(i + 1) * FH],
                                lhsT=xt8[:, t * P:(t + 1) * P],
                                rhs=w1_t[:], start=True, stop=True,
                            )
                        dst_sl = st8[:, bk * BPB * FH:(bk + 1) * BPB * FH]
                        if bk % 2 == 0:
                            nc.scalar.activation(
                                out=dst_sl, in_=hp[:],
                                func=mybir.ActivationFunctionType.Copy,
                            )
                        else:
                            nc.vector.tensor_copy(out=dst_sl, in_=hp[:])
                    nc.sync.dma_start(
                        out=table1[
                            g8 * GT * P:(g8 + 1) * GT * P, :FH
                        ].rearrange("(m p) f -> p m f", p=P),
                        in_=st8[:].rearrange("p (m f) -> p m f", m=GT),
                    )

            (None if os.environ.get("BG_NOBAR") == "1"
                 else tc.strict_bb_all_engine_barrier())

            # ---- aggregation over edges ----
            def build_sel(spool, b):
                """One-hot selector for block b; split across DVE/Pool."""
                s_t = spool.tile([P, P], F16, tag="s")
                eng = (
                    nc.gpsimd
                    if POOL_SEL > 0 and sel_cnt[0] % POOL_SEL == POOL_SEL - 1
                    else nc.vector
                )
                sel_cnt[0] += 1
                eng.tensor_scalar(
                    out=s_t[:], in0=iota_t[:],
                    scalar1=col_t[:, b:b + 1], scalar2=None,
                    op0=mybir.AluOpType.is_equal,
                )
                return s_t

            def agg_layer(layer):
                tbl = table1 if layer == 1 else table2
                FA = FH if layer == 1 else FO
                nchunks = len(cfg.chunks)
                SB = int(os.environ.get("BG_SBUF", "16"))
                with (
                    tc.tile_pool(name=f"msg{layer}", bufs=MBUF) as mpool,
                    tc.tile_pool(name=f"s{layer}", bufs=SB) as spool,
                    tc.tile_pool(name=f"work{layer}", bufs=3 + BUF) as wpool,
                    tc.tile_pool(name=f"ps{layer}", bufs=5, space="PSUM") as ppool,
                    tc.tile_pool(name=f"ps{layer}b", bufs=2, space="PSUM") as qpool,
                ):
                    def issue_gathers(ci):
                        sc0 = cfg.chunk_slot0[ci]
                        j0, j1 = cfg.chunks[ci]
                        cslots = sum(
                            int(cfg.nblk[j, w]) * P
                            for j in range(j0, j1) for w in range(NW)
                        )
                        ckb = cslots // P
                        msg = mpool.tile([P, ckb * FA], F16, tag="msg")
                        for (w, sw0, nw_slots) in cfg.gathers[ci]:
                            bw0 = (sw0 - sc0) // P
                            nbw = nw_slots // P
                            raw_gather(
                                nc,
                                out_ap=msg[
                                    :, bw0 * FA:(bw0 + nbw) * FA
                                ].rearrange("p (b f) -> p b f", f=FA),
                                in_ap=tbl[
                                    w * WIN: w * WIN + cfg.win_rows[w], :FA
                                ],
                                idxs_ap=idx_t[
                                    :, sw0 // 16: (sw0 + nw_slots) // 16
                                ],
                                num_idxs=nw_slots,
                                elem_size=FA,
                            )
                        return msg

                    msgs = {}
                    for ci0 in range(min(MBUF - 1, nchunks)):
                        msgs[ci0] = issue_gathers(ci0)
                    for ci, (j0, j1) in enumerate(cfg.chunks):
                        nxt = ci + MBUF - 1
                        if nxt < nchunks:
                            msgs[nxt] = issue_gathers(nxt)
                        msg = msgs.pop(ci)
                        sc0 = cfg.chunk_slot0[ci]
                        if layer == 1:
                            st2big = wpool.tile([P, (j1 - j0) * FO], F16, tag="st2b")
                        else:
                            otbig = wpool.tile([P, (j1 - j0) * FO], F32, tag="otb")
                            o1big = wpool.tile([P, (j1 - j0) * FO], F32, tag="o1b")
                        for j in range(j0, j1):
                            blocks = cfg.blocks_of_tile[j]
                            nb = len(blocks)
                            ti = j - j0
                            if layer == 1:
                                gt = ppool.tile([FA, P], F32, tag="gt")
                                for i, b in enumerate(blocks):
                                    bl = b - sc0 // P
                                    s_t = build_sel(spool, b)
                                    nc.tensor.matmul(
                                        out=gt[:],
                                        lhsT=msg[:, bl * FA:(bl + 1) * FA],
                                        rhs=s_t[:],
                                        start=(i == 0),
                                        stop=(i == nb - 1 and not cfg.use_b1),
                                    )
                                if cfg.use_b1:
                                    nc.tensor.matmul(
                                        out=gt[:],
                                        lhsT=b1_t[:, :],
                                        rhs=sqd_t[:, j * P:(j + 1) * P],
                                        start=False,
                                        stop=True,
                                    )
                                r1 = wpool.tile([FH, P], F16, tag="r1")
                                nc.scalar.activation(
                                    out=r1[:], in_=gt[:],
                                    func=mybir.ActivationFunctionType.Relu,
                                )
                                h2p = qpool.tile([P, FO], F32, tag="h2")
                                nc.tensor.matmul(
                                    out=h2p[:], lhsT=r1[:], rhs=w2_t[:],
                                    start=True, stop=True,
                                )
                                nc.scalar.activation(
                                    out=st2big[:, ti * FO:(ti + 1) * FO],
                                    in_=h2p[:],
                                    func=mybir.ActivationFunctionType.Copy,
                                    scale=dinv_gsq[:, j:j + 1],
                                )
                            else:
                                # flipped: G[128 dest, FO] += S.T @ msg
                                gt2 = ppool.tile([P, FO], F32, tag="gt2")
                                for i, b in enumerate(blocks):
                                    bl = b - sc0 // P
                                    s_t = build_sel(spool, b)
                                    nc.tensor.matmul(
                                        out=gt2[:],
                                        lhsT=s_t[:],
                                        rhs=msg[:, bl * FA:(bl + 1) * FA],
                                        start=(i == 0),
                                        stop=(i == nb - 1),
                                    )
                                o1 = o1big[:, ti * FO:(ti + 1) * FO]
                                nc.scalar.activation(
                                    out=o1, in_=gt2[:],
                                    func=mybir.ActivationFunctionType.Copy,
                                    scale=dinv_g[:, j:j + 1],
                                )
                                if cfg.use_b2:
                                    nc.vector.tensor_tensor(
                                        out=o1, in0=o1, in1=b2_t[:],
                                        op=mybir.AluOpType.add,
                                    )
                        if layer == 2:
                            # batched log_softmax over the chunk's tiles
                            nt = j1 - j0
                            o3 = o1big[:].rearrange("p (t f) -> p t f", t=nt)
                            nm4 = wpool.tile([P, nt], F32, tag="nm4")
                            nc.vector.tensor_reduce(
                                out=nm4[:], in_=o3,
                                axis=mybir.AxisListType.X,
                                op=mybir.AluOpType.max, negate=True,
                            )
                            nm4b = nm4[:].rearrange(
                                "p (t one) -> p t one", one=1
                            ).to_broadcast([P, nt, FO])
                            o2b = wpool.tile([P, nt * FO], F32, tag="o2b")
                            nc.vector.tensor_tensor(
                                out=o2b[:].rearrange("p (t f) -> p t f", t=nt),
                                in0=o3, in1=nm4b, op=mybir.AluOpType.add,
                            )
                            e4 = wpool.tile([P, nt * FO], F32, tag="e4")
                            nc.scalar.activation(
                                out=e4[:], in_=o2b[:],
                                func=mybir.ActivationFunctionType.Exp,
                            )
                            ss4 = wpool.tile([P, nt], F32, tag="ss4")
                            nc.vector.tensor_reduce(
                                out=ss4[:],
                                in_=e4[:].rearrange("p (t f) -> p t f", t=nt),
                                axis=mybir.AxisListType.X,
                                op=mybir.AluOpType.add,
                            )
                            ls4 = wpool.tile([P, nt], F32, tag="ls4")
                            nc.scalar.activation(
                                out=ls4[:], in_=ss4[:],
                                func=mybir.ActivationFunctionType.Ln,
                            )
                            ls4b = ls4[:].rearrange(
                                "p (t one) -> p t one", one=1
                            ).to_broadcast([P, nt, FO])
                            nc.vector.tensor_tensor(
                                out=otbig[:].rearrange("p (t f) -> p t f", t=nt),
                                in0=o2b[:].rearrange("p (t f) -> p t f", t=nt),
                                in1=ls4b, op=mybir.AluOpType.subtract,
                            )
                        if layer == 1:
                            nc.sync.dma_start(
                                out=t2c[
                                    j0 * P:j1 * P, :
                                ].rearrange("(t p) f -> p t f", p=P),
                                in_=st2big[:].rearrange(
                                    "p (t f) -> p t f", t=j1 - j0
                                ),
                            )
                        else:
                            nc.sync.dma_start(
                                out=out[
                                    j0 * P:j1 * P, :
                                ].rearrange("(t p) f -> p t f", p=P),
                                in_=otbig[:].rearrange(
                                    "p (t f) -> p t f", t=j1 - j0
                                ),
                            )

            if stage != "b":
                agg_layer(1)

            if stage in ("full", "nocoll"):
                (None if os.environ.get("BG_NOBAR") == "1"
                 else tc.strict_bb_all_engine_barrier())
                if stage == "full":
                    nc.gpsimd.collective_compute(
                        "AllGather",
                        mybir.AluOpType.bypass,
                        replica_groups=[list(range(NCORES))],
                        ins=[t2c[:, :]],
                        outs=[t2full[:, :]],
                    )
                # expand compact [PAD_N, FO] into strided table2[:, :FO]
                src_t = t2full if stage == "full" else t2c
                nrow = PAD_N if stage == "full" else SHARD_N
                with tc.tile_pool(name="expand", bufs=3) as epool:
                    ET = 64  # tiles per expand group
                    for g in range(0, nrow // P, ET):
                        ge = min(g + ET, nrow // P)
                        ex = epool.tile([P, (ge - g) * FO], F16, tag="ex")
                        nc.sync.dma_start(
                            out=ex[:],
                            in_=src_t[g * P:ge * P, :].rearrange(
                                "(t p) f -> p t f", p=P
                            ),
                        )
                        nc.sync.dma_start(
                            out=table2[g * P:ge * P, :FO].rearrange(
                                "(t p) f -> p t f", p=P
                            ),
                            in_=ex[:].rearrange("p (t f) -> p t f", t=ge - g),
                        )
                (None if os.environ.get("BG_NOBAR") == "1"
                 else tc.strict_bb_all_engine_barrier())
                agg_layer(2)

    nc.compile()
    return nc


_CACHE = {}
TRACE = False
LAST = None


def kernel(x, edge_index, W1, b1, W2, b2):
    global LAST
    x = np.asarray(x)
    N = x.shape[0]
    cfg, in_maps = prep(x, edge_index, W1, b1, W2, b2)
    key = (
        N, cfg.FIN, cfg.FH, cfg.FO, cfg.SLOTS, cfg.use_b1, cfg.use_b2,
        tuple(cfg.nblk.reshape(-1).tolist()),
    )
    if key not in _CACHE:
        _CACHE[key] = build_program(cfg)
    nc = _CACHE[key]
    try:
        res = run_bass_kernel_spmd(
            nc, in_maps, core_ids=list(range(NCORES)), trace=TRACE
        )
    except Exception:
        # transient device wedge (NRT_EXEC_UNIT_UNRECOVERABLE) -- retry once
        import time as _time
        _time.sleep(10)
        res = run_bass_kernel_spmd(
            nc, in_maps, core_ids=list(range(NCORES)), trace=TRACE
        )
    LAST = res
    outs = [res.results[k]["out"] for k in range(NCORES)]
    full = np.concatenate(outs, axis=0)[:N]
    return full.astype(np.float32)



# revision 12
# speedup vs baseline: 1.2408x; 1.2408x over previous
"""Trainium2 Bass kernel for a 2-layer GCN (nn_ClusterGNN):
    h1 = relu(gcn_conv(x, W1, b1)); out = log_softmax(gcn_conv(h1, W2, b2))

Strategy (8 NeuronCores, dest-node sharded):
  - The GCN symmetric norm factorizes: msg(r->c) = dinv[r]*dinv[c]*h[r].
    dinv[src] is folded into x on the host, the dinv[dst] factor is
    deferred into downstream per-partition scales, so edge aggregation
    is a pure unweighted segment-sum.
  - Self-loops are kept OUT of the edge stream (they would force an
    extra padding block in nearly every (tile, window) group, since
    each core's self-loops land in a different window).  The self
    contribution is added per dest tile with a constant identity
    selector matmul: layer 1 uses self tiles recomputed from a
    per-core xT_self input; layer 2 reuses the core's own t2c tiles
    kept resident in SBUF.
  - Segment-sum runs on the tensor engine: per 128-edge block, a one-hot
    selector S[e, d] = (col_local[e] == d) is built with one
    tensor_scalar(is_equal) in fp16, then layer 1 accumulates
    G^T[f, 128] += msg[128, F].T @ S[128, 128] and layer 2 accumulates
    G[128, F] += S[128, 128].T @ msg[128, F].
  - Edge messages come from raw InstDMAGatherAnt (fp16 rows) out of a
    per-core replicated packed table; int16 indices are window-local.
    Window boundaries are chosen per dataset by a small DP to minimize
    total padded blocks (max-over-cores ceil(cnt/128) per group).
    Gathers are issued one chunk ahead of compute.
  - Layer-2 table  t2full = dinv^2 * (relu(G1) @ W2)  is AllGathered in
    per-chunk pieces as layer 1 produces them (overlapping the
    collective with compute) and gathered from directly (packed rows).
"""

import os
import sys

if "/opt/trn_rl_repo" not in sys.path:
    sys.path.insert(0, "/opt/trn_rl_repo")

import numpy as np

from concourse import bass, bacc, mybir, tile
from concourse.bass_utils import run_bass_kernel_spmd

P = 128
NCORES = 8
WINMAX = 32768
TROW = 128   # table row stride in fp16 elems (= 256B, HW gather stride)
CHUNK_T = int(os.environ.get("BG_CHUNK", "14"))  # dest tiles per gather chunk
POOL_SEL = int(os.environ.get("BG_POOLSEL", "0"))  # every POOL_SEL-th selector on Pool
ACT_SEL = int(os.environ.get("BG_ACTSEL", "0"))  # every ACT_SEL-th selector on Act
MBUF = int(os.environ.get("BG_MBUF", "2"))       # msg tile buffers (chunks in flight)

F32 = mybir.dt.float32
F16 = mybir.dt.float16
I16 = mybir.dt.int16


def cdiv(a, b):
    return -(-a // b)


class Cfg:
    pass


def raw_gather(nc, out_ap, in_ap, idxs_ap, num_idxs, elem_size):
    """InstDMAGatherAnt with arbitrary elem_size."""
    gp = nc.gpsimd
    _in_ap = gp.lower_ap_dma(in_ap, for_custom_bir_dma=True)
    _idxs_ap = gp.lower_ap(idxs_ap)
    _out_ap = gp.lower_ap(out_ap)
    return gp.add_instruction(
        mybir.InstDMAGatherAnt(
            name=nc.get_next_instruction_name(),
            ins=[*_in_ap, _idxs_ap, gp.lower_val_access(gp.to_reg(num_idxs))],
            outs=[_out_ap],
            transpose=False,
            num_idxs=num_idxs,
            elem_size=elem_size,
            stride_bytes_256=1,
            gen_mode=0,
            single_packet=False,
            queue_num=0,
            sbuf_tokens_per_rank=0,
            sbuf_free_dim_per_rank=0,
            sbuf_free_dim_pad_per_rank=0,
            sbuf_byte_offset=0,
        )
    )


def _opt_windows(row, col, NT, SHARD_T, PAD_N, nwin=4, bin_sz=1024):
    """DP window partition minimizing total padded blocks
    (sum over (tile, window) of max-over-cores ceil(cnt/128))."""
    nbin = PAD_N // bin_sz
    tile_of = col // P
    binr = row // bin_sz
    C = np.bincount(tile_of * nbin + binr, minlength=NT * nbin).reshape(
        NCORES, SHARD_T, nbin
    )
    CC = np.concatenate(
        [np.zeros((NCORES, SHARD_T, 1), dtype=np.int64), C.cumsum(axis=2)], axis=2
    )
    maxspan = WINMAX // bin_sz
    INF = 1 << 40
    blk = np.full((nbin + 1, nbin + 1), INF, dtype=np.int64)
    for b0 in range(nbin + 1):
        b1hi = min(nbin, b0 + maxspan)
        if b0 >= b1hi:
            continue
        cnt = CC[:, :, b0 + 1:b1hi + 1] - CC[:, :, b0:b0 + 1]
        blk[b0, b0 + 1:b1hi + 1] = (-(-cnt // P)).max(axis=0).sum(axis=0)
    f = np.full((nwin + 1, nbin + 1), INF, dtype=np.int64)
    pre = np.zeros((nwin + 1, nbin + 1), dtype=np.int64)
    f[0, 0] = 0
    for k in range(1, nwin + 1):
        for b in range(1, nbin + 1):
            lo = max(0, b - maxspan)
            cand = f[k - 1, lo:b] + blk[lo:b, b]
            i = int(np.argmin(cand))
            f[k, b] = cand[i]
            pre[k, b] = lo + i
    b = nbin
    bounds = [PAD_N]
    for k in range(nwin, 0, -1):
        b = int(pre[k, b])
        bounds.append(b * bin_sz)
    bounds.reverse()
    assert bounds[0] == 0
    return bounds


def prep(x, edge_index, W1, b1, W2, b2):
    """Host-side layout prep (shard, sort, window-group, pad)."""
    x = np.asarray(x, dtype=np.float32)
    W1 = np.asarray(W1, dtype=np.float32)
    b1 = np.asarray(b1, dtype=np.float32)
    W2 = np.asarray(W2, dtype=np.float32)
    b2 = np.asarray(b2, dtype=np.float32)

    N, FIN = x.shape
    FH = W1.shape[1]
    FO = W2.shape[1]

    cfg = Cfg()
    cfg.N, cfg.FIN, cfg.FH, cfg.FO = N, FIN, FH, FO
    SHARD_T = cdiv(N, P * NCORES)
    NT = SHARD_T * NCORES
    PAD_N = NT * P
    SHARD_N = SHARD_T * P
    cfg.SHARD_T, cfg.NT, cfg.PAD_N, cfg.SHARD_N = SHARD_T, NT, PAD_N, SHARD_N
    cfg.use_b1 = bool(np.any(b1))
    cfg.use_b2 = bool(np.any(b2))

    row = np.asarray(edge_index[0], dtype=np.int64)
    col = np.asarray(edge_index[1], dtype=np.int64)
    # NO self loops in the edge stream (handled via identity selector)

    # chunks of dest tiles
    chunks = []
    j = 0
    while j < SHARD_T:
        chunks.append((j, min(j + CHUNK_T, SHARD_T)))
        j += CHUNK_T
    cfg.chunks = chunks

    # pi: chunk-major node permutation so per-chunk AllGather outputs are
    # contiguous.  pos(core k, local tile j, p) =
    #   part_base(chunk(j)) + k*rows(chunk) + (j-j0)*P + p
    # Both tables (table1, t2p) and all gather windows use this order.
    nodes_pi = np.empty(PAD_N, dtype=np.int64)
    part_base = []
    s = 0
    for (j0, j1) in chunks:
        part_base.append(s)
        rows = (j1 - j0) * P
        for k in range(NCORES):
            nodes_pi[s:s + rows] = k * SHARD_N + np.arange(j0 * P, j1 * P)
            s += rows
    assert s == PAD_N
    cfg.part_base = part_base
    posn = np.empty(PAD_N, dtype=np.int64)
    posn[nodes_pi] = np.arange(PAD_N)

    bounds = _opt_windows(posn[row], col, NT, SHARD_T, PAD_N)
    NW = len(bounds) - 1
    cfg.NW = NW
    cfg.win_bounds = bounds
    cfg.win_rows = [bounds[w + 1] - bounds[w] for w in range(NW)]

    # sort edges by (dest tile, window, dst)
    tile_of = col // P
    win_of = np.searchsorted(np.asarray(bounds[1:-1]), posn[row], side="right")
    key = (tile_of * NW + win_of) * np.int64(PAD_N) + col
    order = np.argsort(key, kind="stable")
    src_s = row[order]
    dst_s = col[order]
    tw_s = (tile_of * NW + win_of)[order]

    # counts per (global tile, window) -> shared nblk (max over cores)
    cnts = np.bincount(tw_s, minlength=NT * NW).reshape(NCORES, SHARD_T, NW)
    nblk = (-(-cnts // P)).max(axis=0)  # [SHARD_T, NW]
    cfg.nblk = nblk

    # slot layout (shared): for chunk: for w: for j in chunk
    slot_start = np.zeros((SHARD_T, NW), dtype=np.int64)
    gathers = []  # per chunk: (w, slot0, nslots)
    blocks_of_tile = [[] for _ in range(SHARD_T)]
    chunk_slot0 = []
    s = 0
    for (j0, j1) in chunks:
        chunk_slot0.append(s)
        glist = []
        for w in range(NW):
            sw0 = s
            for j in range(j0, j1):
                slot_start[j, w] = s
                nb = int(nblk[j, w])
                blocks_of_tile[j].extend(range(s // P, s // P + nb))
                s += nb * P
            if s > sw0:
                glist.append((w, sw0, s - sw0))
        gathers.append(glist)
    SLOTS = s
    NB = SLOTS // P
    cfg.SLOTS, cfg.NB = SLOTS, NB
    cfg.gathers = gathers
    cfg.chunk_slot0 = chunk_slot0
    cfg.blocks_of_tile = blocks_of_tile

    # degree (incl self-loop) -> dinv, computed on host
    deg_edges = np.bincount(col, minlength=PAD_N).astype(np.float64)
    deg = deg_edges + 1.0
    deg[N:] = 1.0
    dinv = (deg ** -0.5).astype(np.float32)
    dinv[N:] = 0.0

    # x^T fp16 pre-scaled by dinv[src], padded, pi-ordered columns
    xT_n = np.zeros((FIN, PAD_N), dtype=np.float16)
    xT_n[:, :N] = (x * dinv[:N, None]).T.astype(np.float16)
    xT_perm = xT_n[:, nodes_pi].copy()

    iota = np.broadcast_to(np.arange(P, dtype=np.float16)[None, :], (P, P)).copy()
    ident = np.eye(P, dtype=np.float16)
    W1h = W1.astype(np.float16)          # [FIN, FH] unpadded
    W2h = W2.astype(np.float16)          # [FH, FO]
    b2rep = np.broadcast_to(b2[None, :], (P, FO)).copy()
    b1r = b1[None, :].copy()

    rp_tiles = np.searchsorted(tw_s, np.arange(NT * NW + 1))

    in_maps = []
    for k in range(NCORES):
        idx_flat = np.zeros(SLOTS, dtype=np.int16)
        col_flat = np.full(SLOTS, -1.0, dtype=np.float32)
        for j in range(SHARD_T):
            t = k * SHARD_T + j
            for w in range(NW):
                a, b = rp_tiles[t * NW + w], rp_tiles[t * NW + w + 1]
                n = b - a
                if n == 0:
                    continue
                s0 = slot_start[j, w]
                idx_flat[s0:s0 + n] = (posn[src_s[a:b]] - bounds[w]).astype(
                    np.int16
                )
                col_flat[s0:s0 + n] = (dst_s[a:b] - t * P).astype(np.float32)
        idx16 = np.tile(idx_flat.reshape(-1, 16).T, (NCORES, 1))
        col_arr = col_flat.reshape(NB, P).T.copy()

        gn = (k * SHARD_N + np.arange(SHARD_N)).reshape(SHARD_T, P).T
        dg = dinv[gn]
        # per-core own-shard x^T (dinv-scaled) for the self-loop tiles
        xT_self = xT_n[:, k * SHARD_N:(k + 1) * SHARD_N].copy()
        in_map = {
            "xT": xT_perm,
            "xT_self": xT_self,
            "w1h": W1h,
            "w2h": W2h,
            "iota": iota,
            "ident": ident,
            "idx16": idx16,
            "colv": col_arr,
            "dinv_g": dg.copy(),
            "dinv_gsq": (dg * dg).copy(),
        }
        if cfg.use_b2:
            in_map["b2rep"] = b2rep
        if cfg.use_b1:
            rn = k * SHARD_N + np.arange(SHARD_N)
            in_map["b1r"] = b1r
            # sqrt(deg) = 1/dinv (safe: pad nodes never relu'd into output)
            sq = np.where(dinv[rn] > 0, 1.0 / np.maximum(dinv[rn], 1e-30), 0.0)
            in_map["sqd_r"] = sq[None, :].astype(np.float32).copy()
        in_maps.append(in_map)

    return cfg, in_maps


def build_program(cfg):
    FIN, FH, FO = cfg.FIN, cfg.FH, cfg.FO
    NT, SHARD_T, PAD_N, SHARD_N = cfg.NT, cfg.SHARD_T, cfg.PAD_N, cfg.SHARD_N
    NW, NB, SLOTS = cfg.NW, cfg.NB, cfg.SLOTS
    WB = cfg.win_bounds

    BUF = int(os.environ.get("BG_BUF", "0"))
    nc = bacc.Bacc(
        "TRN2", target_bir_lowering=False, debug=False, num_devices=NCORES
    )

    xT_in = nc.dram_tensor("xT", [FIN, PAD_N], F16, kind="ExternalInput").ap()
    xT_self_in = nc.dram_tensor(
        "xT_self", [FIN, SHARD_N], F16, kind="ExternalInput"
    ).ap()
    w1h_in = nc.dram_tensor("w1h", [FIN, FH], F16, kind="ExternalInput").ap()
    w2h_in = nc.dram_tensor("w2h", [FH, FO], F16, kind="ExternalInput").ap()
    iota_in = nc.dram_tensor("iota", [P, P], F16, kind="ExternalInput").ap()
    ident_in = nc.dram_tensor("ident", [P, P], F16, kind="ExternalInput").ap()
    idx_in = nc.dram_tensor("idx16", [P, SLOTS // 16], I16, kind="ExternalInput").ap()
    col_in = nc.dram_tensor("colv", [P, NB], F32, kind="ExternalInput").ap()
    dinv_g_in = nc.dram_tensor("dinv_g", [P, SHARD_T], F32, kind="ExternalInput").ap()
    dinv_gsq_in = nc.dram_tensor(
        "dinv_gsq", [P, SHARD_T], F32, kind="ExternalInput"
    ).ap()
    if cfg.use_b2:
        b2rep_in = nc.dram_tensor("b2rep", [P, FO], F32, kind="ExternalInput").ap()
    if cfg.use_b1:
        b1r_in = nc.dram_tensor("b1r", [1, FH], F32, kind="ExternalInput").ap()
        sqd_in = nc.dram_tensor("sqd_r", [1, SHARD_N], F32, kind="ExternalInput").ap()

    table1 = nc.dram_tensor("table1", [PAD_N, TROW], F16, kind="Internal").ap()
    table2 = nc.dram_tensor("table2", [PAD_N, TROW], F16, kind="Internal").ap()
    t2c = nc.dram_tensor("t2c", [SHARD_N, FO], F16, kind="Internal").ap()
    t2full = nc.dram_tensor(
        "t2full", [PAD_N, FO], F16, kind="Internal", addr_space="Shared"
    ).ap()
    out = nc.dram_tensor("out", [SHARD_N, FO], F32, kind="ExternalOutput").ap()

    stage = os.environ.get("BASSGNN_STAGE", "full")
    sel_cnt = [0]

    with tile.TileContext(nc) as tc:
        with tc.tile_pool(name="const", bufs=1) as cpool:
            w1_t = cpool.tile([FIN, FH], F16)
            nc.sync.dma_start(out=w1_t[:], in_=w1h_in[:, :])
            w2_t = cpool.tile([FH, FO], F16)
            nc.sync.dma_start(out=w2_t[:], in_=w2h_in[:, :])
            iota_t = cpool.tile([P, P], F16)
            nc.sync.dma_start(out=iota_t[:], in_=iota_in[:, :])
            ident_t = cpool.tile([P, P], F16)
            nc.sync.dma_start(out=ident_t[:], in_=ident_in[:, :])
            col_t = cpool.tile([P, NB], F32)
            nc.sync.dma_start(out=col_t[:], in_=col_in[:, :])
            idx_t = cpool.tile([P, SLOTS // 16], I16)
            nc.sync.dma_start(out=idx_t[:], in_=idx_in[:, :])
            dinv_g = cpool.tile([P, SHARD_T], F32)
            nc.sync.dma_start(out=dinv_g[:], in_=dinv_g_in[:, :])
            dinv_gsq = cpool.tile([P, SHARD_T], F32)
            nc.sync.dma_start(out=dinv_gsq[:], in_=dinv_gsq_in[:, :])
            if cfg.use_b2:
                b2_t = cpool.tile([P, FO], F32)
                nc.sync.dma_start(out=b2_t[:], in_=b2rep_in[:, :])
            if cfg.use_b1:
                b1_t = cpool.tile([1, FH], F32)
                nc.sync.dma_start(out=b1_t[:], in_=b1r_in[:, :])
                sqd_t = cpool.tile([1, SHARD_N], F32)
                nc.sync.dma_start(out=sqd_t[:], in_=sqd_in[:, :])
            # persistent self tiles: layer-1 (own-shard table1 rows) and
            # layer-2 (own t2c tiles, filled during layer-1 aggregation)
            self1_t = cpool.tile([P, SHARD_T * FH], F16)
            self2_t = cpool.tile([P, SHARD_T * FO], F16)

            # ---- Phase B: table1 = (dinv*x) @ W1, full table per core ----
            with (
                tc.tile_pool(name="phb", bufs=3 + BUF) as bpool,
                tc.tile_pool(name="phb_st", bufs=3 + BUF) as stpool,
                tc.tile_pool(name="phb_ps", bufs=4, space="PSUM") as bpsum,
            ):
                GT = 16
                assert NT % GT == 0
                BPB = 512 // FH   # matmul tiles per PSUM bank
                for g8 in range(NT // GT):
                    xt8 = bpool.tile([P, GT * P], F16, tag="xt")
                    nc.sync.dma_start(
                        out=xt8[:], in_=xT_in[:, g8 * GT * P:(g8 + 1) * GT * P]
                    )
                    st8 = stpool.tile([P, GT * FH], F16, tag="st")
                    for bk in range(GT // BPB):
                        hp = bpsum.tile([P, BPB * FH], F32, tag="hp")
                        for i in range(BPB):
                            t = bk * BPB + i
                            nc.tensor.matmul(
                                out=hp[:, i * FH:(i + 1) * FH],
                                lhsT=xt8[:, t * P:(t + 1) * P],
                                rhs=w1_t[:], start=True, stop=True,
                            )
                        dst_sl = st8[:, bk * BPB * FH:(bk + 1) * BPB * FH]
                        if bk % 2 == 0:
                            nc.scalar.activation(
                                out=dst_sl, in_=hp[:],
                                func=mybir.ActivationFunctionType.Copy,
                            )
                        else:
                            nc.vector.tensor_copy(out=dst_sl, in_=hp[:])
                    nc.sync.dma_start(
                        out=table1[
                            g8 * GT * P:(g8 + 1) * GT * P, :FH
                        ].rearrange("(m p) f -> p m f", p=P),
                        in_=st8[:].rearrange("p (m f) -> p m f", m=GT),
                    )
                # self tiles for layer 1: own-shard (dinv*x) @ W1, kept in SBUF
                sg = 0
                while sg < SHARD_T:
                    gt_n = min(GT, SHARD_T - sg)
                    xs = bpool.tile([P, gt_n * P], F16, tag="xt")
                    nc.sync.dma_start(
                        out=xs[:], in_=xT_self_in[:, sg * P:(sg + gt_n) * P]
                    )
                    for bk in range(cdiv(gt_n, BPB)):
                        nbk = min(BPB, gt_n - bk * BPB)
                        hp = bpsum.tile([P, nbk * FH], F32, tag="hp")
                        for i in range(nbk):
                            t = bk * BPB + i
                            nc.tensor.matmul(
                                out=hp[:, i * FH:(i + 1) * FH],
                                lhsT=xs[:, t * P:(t + 1) * P],
                                rhs=w1_t[:], start=True, stop=True,
                            )
                        dst_sl = self1_t[
                            :, (sg + bk * BPB) * FH:(sg + bk * BPB + nbk) * FH
                        ]
                        if bk % 2 == 0:
                            nc.scalar.activation(
                                out=dst_sl, in_=hp[:],
                                func=mybir.ActivationFunctionType.Copy,
                            )
                        else:
                            nc.vector.tensor_copy(out=dst_sl, in_=hp[:])
                    sg += gt_n

            (None if os.environ.get("BG_NOBAR") == "1"
                 else tc.strict_bb_all_engine_barrier())

            # ---- aggregation over edges ----
            def build_sel(spool, b):
                """One-hot selector for block b."""
                s_t = spool.tile([P, P], F16, tag="s")
                eng = (
                    nc.gpsimd
                    if POOL_SEL > 0 and sel_cnt[0] % POOL_SEL == POOL_SEL - 1
                    else nc.vector
                )
                sel_cnt[0] += 1
                eng.tensor_scalar(
                    out=s_t[:], in0=iota_t[:],
                    scalar1=col_t[:, b:b + 1], scalar2=None,
                    op0=mybir.AluOpType.is_equal,
                )
                return s_t

            def agg_layer(layer):
                FA = FH if layer == 1 else FO
                nchunks = len(cfg.chunks)
                SB = int(os.environ.get("BG_SBUF", "16"))
                with (
                    tc.tile_pool(name=f"msg{layer}", bufs=MBUF) as mpool,
                    tc.tile_pool(name=f"s{layer}", bufs=SB) as spool,
                    tc.tile_pool(name=f"work{layer}", bufs=3 + BUF) as wpool,
                    tc.tile_pool(name=f"ps{layer}", bufs=5, space="PSUM") as ppool,
                    tc.tile_pool(name=f"ps{layer}b", bufs=2, space="PSUM") as qpool,
                ):
                    def issue_gathers(ci):
                        sc0 = cfg.chunk_slot0[ci]
                        j0, j1 = cfg.chunks[ci]
                        cslots = sum(
                            int(cfg.nblk[j, w]) * P
                            for j in range(j0, j1) for w in range(NW)
                        )
                        ckb = cslots // P
                        msg = mpool.tile([P, ckb * FA], F16, tag="msg")
                        for (w, sw0, nw_slots) in cfg.gathers[ci]:
                            bw0 = (sw0 - sc0) // P
                            nbw = nw_slots // P
                            if layer == 1:
                                src_ap = table1[WB[w]:WB[w + 1], :FA]
                            else:
                                src_ap = table2[WB[w]:WB[w + 1], :FA]
                            raw_gather(
                                nc,
                                out_ap=msg[
                                    :, bw0 * FA:(bw0 + nbw) * FA
                                ].rearrange("p (b f) -> p b f", f=FA),
                                in_ap=src_ap,
                                idxs_ap=idx_t[
                                    :, sw0 // 16: (sw0 + nw_slots) // 16
                                ],
                                num_idxs=nw_slots,
                                elem_size=FA,
                            )
                        return msg

                    msgs = {}
                    for ci0 in range(min(MBUF - 1, nchunks)):
                        msgs[ci0] = issue_gathers(ci0)
                    for ci, (j0, j1) in enumerate(cfg.chunks):
                        nxt = ci + MBUF - 1
                        if nxt < nchunks:
                            msgs[nxt] = issue_gathers(nxt)
                        msg = msgs.pop(ci)
                        sc0 = cfg.chunk_slot0[ci]
                        if layer == 1:
                            st2big = self2_t[:, j0 * FO:j1 * FO]
                        else:
                            otbig = wpool.tile([P, (j1 - j0) * FO], F32, tag="otb")
                            o1big = wpool.tile([P, (j1 - j0) * FO], F32, tag="o1b")
                        for j in range(j0, j1):
                            blocks = cfg.blocks_of_tile[j]
                            nb = len(blocks)
                            ti = j - j0
                            if layer == 1:
                                gt = ppool.tile([FA, P], F32, tag="gt")
                                noself = os.environ.get("BG_NOSELF") == "1"
                                # self-loop contribution: table1 row d for
                                # dest d = self1_t tile j (identity selector)
                                if not noself:
                                    nc.tensor.matmul(
                                        out=gt[:],
                                        lhsT=self1_t[:, j * FH:(j + 1) * FH],
                                        rhs=ident_t[:],
                                        start=True, stop=False,
                                    )
                                for i, b in enumerate(blocks):
                                    bl = b - sc0 // P
                                    s_t = build_sel(spool, b)
                                    nc.tensor.matmul(
                                        out=gt[:],
                                        lhsT=msg[:, bl * FA:(bl + 1) * FA],
                                        rhs=s_t[:],
                                        start=(noself and i == 0),
                                        stop=(i == nb - 1 and not cfg.use_b1),
                                    )
                                if cfg.use_b1:
                                    nc.tensor.matmul(
                                        out=gt[:],
                                        lhsT=b1_t[:, :],
                                        rhs=sqd_t[:, j * P:(j + 1) * P],
                                        start=False,
                                        stop=True,
                                    )
                                r1 = wpool.tile([FH, P], F16, tag="r1")
                                nc.scalar.activation(
                                    out=r1[:], in_=gt[:],
                                    func=mybir.ActivationFunctionType.Relu,
                                )
                                h2p = qpool.tile([P, FO], F32, tag="h2")
                                nc.tensor.matmul(
                                    out=h2p[:], lhsT=r1[:], rhs=w2_t[:],
                                    start=True, stop=True,
                                )
                                nc.scalar.activation(
                                    out=st2big[:, ti * FO:(ti + 1) * FO],
                                    in_=h2p[:],
                                    func=mybir.ActivationFunctionType.Copy,
                                    scale=dinv_gsq[:, j:j + 1],
                                )
                            else:
                                # flipped: G[128 dest, FO] += S.T @ msg
                                gt2 = ppool.tile([P, FO], F32, tag="gt2")
                                noself = os.environ.get("BG_NOSELF") == "1"
                                # self-loop: t2full row d for dest d =
                                # own t2c tile j (kept in SBUF)
                                if not noself:
                                    nc.tensor.matmul(
                                        out=gt2[:],
                                        lhsT=ident_t[:],
                                        rhs=self2_t[:, j * FO:(j + 1) * FO],
                                        start=True, stop=False,
                                    )
                                for i, b in enumerate(blocks):
                                    bl = b - sc0 // P
                                    s_t = build_sel(spool, b)
                                    nc.tensor.matmul(
                                        out=gt2[:],
                                        lhsT=s_t[:],
                                        rhs=msg[:, bl * FA:(bl + 1) * FA],
                                        start=(noself and i == 0),
                                        stop=(i == nb - 1),
                                    )
                                o1 = o1big[:, ti * FO:(ti + 1) * FO]
                                nc.scalar.activation(
                                    out=o1, in_=gt2[:],
                                    func=mybir.ActivationFunctionType.Copy,
                                    scale=dinv_g[:, j:j + 1],
                                )
                                if cfg.use_b2:
                                    nc.vector.tensor_tensor(
                                        out=o1, in0=o1, in1=b2_t[:],
                                        op=mybir.AluOpType.add,
                                    )
                        if layer == 2:
                            # batched log_softmax over the chunk's tiles
                            nt = j1 - j0
                            o3 = o1big[:].rearrange("p (t f) -> p t f", t=nt)
                            nm4 = wpool.tile([P, nt], F32, tag="nm4")
                            nc.vector.tensor_reduce(
                                out=nm4[:], in_=o3,
                                axis=mybir.AxisListType.X,
                                op=mybir.AluOpType.max, negate=True,
                            )
                            nm4b = nm4[:].rearrange(
                                "p (t one) -> p t one", one=1
                            ).to_broadcast([P, nt, FO])
                            o2b = wpool.tile([P, nt * FO], F32, tag="o2b")
                            nc.vector.tensor_tensor(
                                out=o2b[:].rearrange("p (t f) -> p t f", t=nt),
                                in0=o3, in1=nm4b, op=mybir.AluOpType.add,
                            )
                            e4 = wpool.tile([P, nt * FO], F32, tag="e4")
                            nc.scalar.activation(
                                out=e4[:], in_=o2b[:],
                                func=mybir.ActivationFunctionType.Exp,
                            )
                            ss4 = wpool.tile([P, nt], F32, tag="ss4")
                            nc.vector.tensor_reduce(
                                out=ss4[:],
                                in_=e4[:].rearrange("p (t f) -> p t f", t=nt),
                                axis=mybir.AxisListType.X,
                                op=mybir.AluOpType.add,
                            )
                            ls4 = wpool.tile([P, nt], F32, tag="ls4")
                            nc.scalar.activation(
                                out=ls4[:], in_=ss4[:],
                                func=mybir.ActivationFunctionType.Ln,
                            )
                            ls4b = ls4[:].rearrange(
                                "p (t one) -> p t one", one=1
                            ).to_broadcast([P, nt, FO])
                            nc.vector.tensor_tensor(
                                out=otbig[:].rearrange("p (t f) -> p t f", t=nt),
                                in0=o2b[:].rearrange("p (t f) -> p t f", t=nt),
                                in1=ls4b, op=mybir.AluOpType.subtract,
                            )
                        if layer == 1:
                            nc.sync.dma_start(
                                out=t2c[
                                    j0 * P:j1 * P, :
                                ].rearrange("(t p) f -> p t f", p=P),
                                in_=st2big[:].rearrange(
                                    "p (t f) -> p t f", t=j1 - j0
                                ),
                            )
                            pb = cfg.part_base[ci]
                            prows = NCORES * (j1 - j0) * P
                            if stage == "full":
                                # chunked AllGather overlapped with compute;
                                # pi layout makes the output contiguous
                                nc.gpsimd.collective_compute(
                                    "AllGather",
                                    mybir.AluOpType.bypass,
                                    replica_groups=[list(range(NCORES))],
                                    ins=[t2c[j0 * P:j1 * P, :]],
                                    outs=[t2full[pb:pb + prows, :]],
                                )
                            # expand this part into the 256B-strided table2
                            # (HW gather rows must be 256B-stride aligned)
                            ex = wpool.tile([P, (prows // P) * FO], F16,
                                            tag="ex")
                            nc.sync.dma_start(
                                out=ex[:],
                                in_=t2full[pb:pb + prows, :].rearrange(
                                    "(p t) f -> p t f", p=P
                                ),
                            )
                            nc.sync.dma_start(
                                out=table2[pb:pb + prows, :FO].rearrange(
                                    "(p t) f -> p t f", p=P
                                ),
                                in_=ex[:].rearrange(
                                    "p (t f) -> p t f", t=prows // P
                                ),
                            )
                        else:
                            nc.sync.dma_start(
                                out=out[
                                    j0 * P:j1 * P, :
                                ].rearrange("(t p) f -> p t f", p=P),
                                in_=otbig[:].rearrange(
                                    "p (t f) -> p t f", t=j1 - j0
                                ),
                            )

            if stage != "b":
                agg_layer(1)

            if stage in ("full", "nocoll"):
                (None if os.environ.get("BG_NOBAR") == "1"
                 else tc.strict_bb_all_engine_barrier())
                agg_layer(2)

    nc.compile()
    return nc


_CACHE = {}
TRACE = False
LAST = None


def kernel(x, edge_index, W1, b1, W2, b2):
    global LAST
    x = np.asarray(x)
    N = x.shape[0]
    cfg, in_maps = prep(x, edge_index, W1, b1, W2, b2)
    key = (
        N, cfg.FIN, cfg.FH, cfg.FO, cfg.SLOTS, cfg.use_b1, cfg.use_b2,
        tuple(cfg.win_bounds),
        tuple(cfg.nblk.reshape(-1).tolist()),
    )
    if key not in _CACHE:
        _CACHE[key] = build_program(cfg)
    nc = _CACHE[key]
    try:
        res = run_bass_kernel_spmd(
            nc, in_maps, core_ids=list(range(NCORES)), trace=TRACE
        )
    except Exception:
        # transient device wedge (NRT_EXEC_UNIT_UNRECOVERABLE) -- retry once
        import time as _time
        _time.sleep(10)
        res = run_bass_kernel_spmd(
            nc, in_maps, core_ids=list(range(NCORES)), trace=TRACE
        )
    LAST = res
    outs = [res.results[k]["out"] for k in range(NCORES)]
    full = np.concatenate(outs, axis=0)[:N]
    return full.astype(np.float32)


# revision 30
# speedup vs baseline: 1.6216x; 1.3069x over previous
"""Trainium2 Bass kernel for a 2-layer GCN (nn_ClusterGNN):
    h1 = relu(gcn_conv(x, W1, b1)); out = log_softmax(gcn_conv(h1, W2, b2))

Strategy (8 NeuronCores, dest-node sharded):
  - The GCN symmetric norm factorizes: msg(r->c) = dinv[r]*dinv[c]*h[r].
    dinv[src] is folded into x on the host, the dinv[dst] factor is
    deferred into downstream per-partition scales, so edge aggregation
    is a pure unweighted segment-sum.
  - Self-loops are kept OUT of the edge stream (they would force an
    extra padding block in nearly every (tile, window) group, since
    each core's self-loops land in a different window).  The self
    contribution is added per dest tile with a constant identity
    selector matmul: layer 1 uses self tiles recomputed from a
    per-core xT_self input; layer 2 reuses the core's own t2c tiles
    kept resident in SBUF.
  - Segment-sum runs on the tensor engine: per 128-edge block, a one-hot
    selector S[e, d] = (col_local[e] == d) is built with one
    tensor_scalar(is_equal) in fp16, then layer 1 accumulates
    G^T[f, 128] += msg[128, F].T @ S[128, 128] and layer 2 accumulates
    G[128, F] += S[128, 128].T @ msg[128, F].
  - Edge messages come from raw InstDMAGatherAnt (fp16 rows) out of a
    per-core replicated packed table; int16 indices are window-local.
    Window boundaries are chosen per dataset by a small DP to minimize
    total padded blocks (max-over-cores ceil(cnt/128) per group).
    Gathers are issued one chunk ahead of compute.
  - Layer-2 table  t2full = dinv^2 * (relu(G1) @ W2)  is AllGathered in
    per-chunk pieces as layer 1 produces them (overlapping the
    collective with compute) and gathered from directly (packed rows).
"""

import os
import sys

if "/opt/trn_rl_repo" not in sys.path:
    sys.path.insert(0, "/opt/trn_rl_repo")

import numpy as np

from concourse import bass, bacc, mybir, tile
from concourse.tile_rust import add_dep_helper
from concourse.bass_utils import run_bass_kernel_spmd

P = 128
NCORES = 8
WINMAX = 32768
TROW = 128   # table row stride in fp16 elems (= 256B, HW gather stride)
CHUNK_T = int(os.environ.get("BG_CHUNK", "6"))  # dest tiles per gather chunk
POOL_SEL = int(os.environ.get("BG_POOLSEL", "0"))  # every POOL_SEL-th selector on Pool
ACT_SEL = int(os.environ.get("BG_ACTSEL", "0"))  # every ACT_SEL-th selector on Act
MBUF = int(os.environ.get("BG_MBUF", "3"))       # msg tile buffers (chunks in flight)

F32 = mybir.dt.float32
F16 = mybir.dt.float16
I16 = mybir.dt.int16


def cdiv(a, b):
    return -(-a // b)


class Cfg:
    pass


def raw_gather(nc, out_ap, in_ap, idxs_ap, num_idxs, elem_size):
    """InstDMAGatherAnt with arbitrary elem_size."""
    gp = nc.gpsimd
    _in_ap = gp.lower_ap_dma(in_ap, for_custom_bir_dma=True)
    _idxs_ap = gp.lower_ap(idxs_ap)
    _out_ap = gp.lower_ap(out_ap)
    return gp.add_instruction(
        mybir.InstDMAGatherAnt(
            name=nc.get_next_instruction_name(),
            ins=[*_in_ap, _idxs_ap, gp.lower_val_access(gp.to_reg(num_idxs))],
            outs=[_out_ap],
            transpose=False,
            num_idxs=num_idxs,
            elem_size=elem_size,
            stride_bytes_256=1,
            gen_mode=0,
            single_packet=False,
            queue_num=0,
            sbuf_tokens_per_rank=0,
            sbuf_free_dim_per_rank=0,
            sbuf_free_dim_pad_per_rank=0,
            sbuf_byte_offset=0,
        )
    )


def _opt_windows(row, col, NT, SHARD_T, PAD_N, nwin=4, bin_sz=1024):
    """DP window partition minimizing total padded blocks
    (sum over (tile, window) of max-over-cores ceil(cnt/128))."""
    nbin = PAD_N // bin_sz
    tile_of = col // P
    binr = row // bin_sz
    C = np.bincount(tile_of * nbin + binr, minlength=NT * nbin).reshape(
        NCORES, SHARD_T, nbin
    )
    CC = np.concatenate(
        [np.zeros((NCORES, SHARD_T, 1), dtype=np.int64), C.cumsum(axis=2)], axis=2
    )
    maxspan = WINMAX // bin_sz
    INF = 1 << 40
    blk = np.full((nbin + 1, nbin + 1), INF, dtype=np.int64)
    for b0 in range(nbin + 1):
        b1hi = min(nbin, b0 + maxspan)
        if b0 >= b1hi:
            continue
        cnt = CC[:, :, b0 + 1:b1hi + 1] - CC[:, :, b0:b0 + 1]
        blk[b0, b0 + 1:b1hi + 1] = (-(-cnt // P)).max(axis=0).sum(axis=0)
    f = np.full((nwin + 1, nbin + 1), INF, dtype=np.int64)
    pre = np.zeros((nwin + 1, nbin + 1), dtype=np.int64)
    f[0, 0] = 0
    for k in range(1, nwin + 1):
        for b in range(1, nbin + 1):
            lo = max(0, b - maxspan)
            cand = f[k - 1, lo:b] + blk[lo:b, b]
            i = int(np.argmin(cand))
            f[k, b] = cand[i]
            pre[k, b] = lo + i
    b = nbin
    bounds = [PAD_N]
    for k in range(nwin, 0, -1):
        b = int(pre[k, b])
        bounds.append(b * bin_sz)
    bounds.reverse()
    assert bounds[0] == 0
    return bounds


def prep(x, edge_index, W1, b1, W2, b2):
    """Host-side layout prep (shard, sort, window-group, pad)."""
    x = np.asarray(x, dtype=np.float32)
    W1 = np.asarray(W1, dtype=np.float32)
    b1 = np.asarray(b1, dtype=np.float32)
    W2 = np.asarray(W2, dtype=np.float32)
    b2 = np.asarray(b2, dtype=np.float32)

    N, FIN = x.shape
    FH = W1.shape[1]
    FO = W2.shape[1]

    cfg = Cfg()
    cfg.N, cfg.FIN, cfg.FH, cfg.FO = N, FIN, FH, FO
    SHARD_T = cdiv(N, P * NCORES)
    NT = SHARD_T * NCORES
    PAD_N = NT * P
    SHARD_N = SHARD_T * P
    cfg.SHARD_T, cfg.NT, cfg.PAD_N, cfg.SHARD_N = SHARD_T, NT, PAD_N, SHARD_N
    cfg.use_b1 = bool(np.any(b1))
    cfg.use_b2 = bool(np.any(b2))

    row = np.asarray(edge_index[0], dtype=np.int64)
    col = np.asarray(edge_index[1], dtype=np.int64)
    # NO self loops in the edge stream (handled via identity selector)

    # chunks of dest tiles
    chunks = []
    j = 0
    while j < SHARD_T:
        chunks.append((j, min(j + CHUNK_T, SHARD_T)))
        j += CHUNK_T
    cfg.chunks = chunks

    # pi: chunk-major node permutation so per-chunk AllGather outputs are
    # contiguous.  pos(core k, local tile j, p) =
    #   part_base(chunk(j)) + k*rows(chunk) + (j-j0)*P + p
    # Both tables (table1, t2p) and all gather windows use this order.
    nodes_pi = np.empty(PAD_N, dtype=np.int64)
    part_base = []
    s = 0
    for (j0, j1) in chunks:
        part_base.append(s)
        rows = (j1 - j0) * P
        for k in range(NCORES):
            nodes_pi[s:s + rows] = k * SHARD_N + np.arange(j0 * P, j1 * P)
            s += rows
    assert s == PAD_N
    cfg.part_base = part_base
    posn = np.empty(PAD_N, dtype=np.int64)
    posn[nodes_pi] = np.arange(PAD_N)

    bounds = _opt_windows(posn[row], col, NT, SHARD_T, PAD_N)
    NW = len(bounds) - 1
    cfg.NW = NW
    cfg.win_bounds = bounds
    cfg.win_rows = [bounds[w + 1] - bounds[w] for w in range(NW)]

    # sort edges by (dest tile, window, dst)
    tile_of = col // P
    win_of = np.searchsorted(np.asarray(bounds[1:-1]), posn[row], side="right")
    key = (tile_of * NW + win_of) * np.int64(PAD_N) + col
    order = np.argsort(key, kind="stable")
    src_s = row[order]
    dst_s = col[order]
    tw_s = (tile_of * NW + win_of)[order]

    # counts per (global tile, window) -> shared nblk (max over cores)
    cnts = np.bincount(tw_s, minlength=NT * NW).reshape(NCORES, SHARD_T, NW)
    nblk = (-(-cnts // P)).max(axis=0)  # [SHARD_T, NW]
    cfg.nblk = nblk

    # slot layout (shared): for chunk: for w: for j in chunk
    slot_start = np.zeros((SHARD_T, NW), dtype=np.int64)
    gathers = []  # per chunk: (w, slot0, nslots)
    blocks_of_tile = [[] for _ in range(SHARD_T)]
    chunk_slot0 = []
    s = 0
    for (j0, j1) in chunks:
        chunk_slot0.append(s)
        glist = []
        for w in range(NW):
            sw0 = s
            for j in range(j0, j1):
                slot_start[j, w] = s
                nb = int(nblk[j, w])
                blocks_of_tile[j].extend(range(s // P, s // P + nb))
                s += nb * P
            if s > sw0:
                glist.append((w, sw0, s - sw0))
        gathers.append(glist)
    SLOTS = s
    NB = SLOTS // P
    cfg.SLOTS, cfg.NB = SLOTS, NB
    cfg.gathers = gathers
    cfg.chunk_slot0 = chunk_slot0
    cfg.blocks_of_tile = blocks_of_tile

    # degree (incl self-loop) -> dinv, computed on host
    deg_edges = np.bincount(col, minlength=PAD_N).astype(np.float64)
    deg = deg_edges + 1.0
    deg[N:] = 1.0
    dinv = (deg ** -0.5).astype(np.float32)
    dinv[N:] = 0.0

    # x^T fp16 pre-scaled by dinv[src], padded, pi-ordered columns
    xT_n = np.zeros((FIN, PAD_N), dtype=np.float16)
    xT_n[:, :N] = (x * dinv[:N, None]).T.astype(np.float16)
    xT_perm = xT_n[:, nodes_pi].copy()

    iota = np.broadcast_to(np.arange(P, dtype=np.float16)[None, :], (P, P)).copy()
    ident = np.eye(P, dtype=np.float16)
    W1h = W1.astype(np.float16)          # [FIN, FH] unpadded
    W2h = W2.astype(np.float16)          # [FH, FO]
    b2rep = np.broadcast_to(b2[None, :], (P, FO)).copy()
    b1r = b1[None, :].copy()

    rp_tiles = np.searchsorted(tw_s, np.arange(NT * NW + 1))

    in_maps = []
    for k in range(NCORES):
        idx_flat = np.zeros(SLOTS, dtype=np.int16)
        col_flat = np.full(SLOTS, -1.0, dtype=np.float32)
        for j in range(SHARD_T):
            t = k * SHARD_T + j
            for w in range(NW):
                a, b = rp_tiles[t * NW + w], rp_tiles[t * NW + w + 1]
                n = b - a
                if n == 0:
                    continue
                s0 = slot_start[j, w]
                idx_flat[s0:s0 + n] = (posn[src_s[a:b]] - bounds[w]).astype(
                    np.int16
                )
                col_flat[s0:s0 + n] = (dst_s[a:b] - t * P).astype(np.float32)
        idx16 = np.tile(idx_flat.reshape(-1, 16).T, (NCORES, 1))
        col_arr = col_flat.reshape(NB, P).T.copy()

        gn = (k * SHARD_N + np.arange(SHARD_N)).reshape(SHARD_T, P).T
        dg = dinv[gn]
        # per-core own-shard x^T (dinv-scaled) for the self-loop tiles
        xT_self = xT_n[:, k * SHARD_N:(k + 1) * SHARD_N].copy()
        in_map = {
            "xT": xT_perm,
            "xT_self": xT_self,
            "negcolv": (-col_arr).copy(),
            "w1h": W1h,
            "w2h": W2h,
            "iota": iota,
            "ident": ident,
            "idx16": idx16,
            "colv": col_arr,
            "dinv_g": dg.copy(),
            "dinv_gsq": (dg * dg).copy(),
        }
        if cfg.use_b2:
            in_map["b2rep"] = b2rep
        if cfg.use_b1:
            rn = k * SHARD_N + np.arange(SHARD_N)
            in_map["b1r"] = b1r
            # sqrt(deg) = 1/dinv (safe: pad nodes never relu'd into output)
            sq = np.where(dinv[rn] > 0, 1.0 / np.maximum(dinv[rn], 1e-30), 0.0)
            in_map["sqd_r"] = sq[None, :].astype(np.float32).copy()
        in_maps.append(in_map)

    return cfg, in_maps


def build_program(cfg):
    FIN, FH, FO = cfg.FIN, cfg.FH, cfg.FO
    NT, SHARD_T, PAD_N, SHARD_N = cfg.NT, cfg.SHARD_T, cfg.PAD_N, cfg.SHARD_N
    NW, NB, SLOTS = cfg.NW, cfg.NB, cfg.SLOTS
    WB = cfg.win_bounds

    BUF = int(os.environ.get("BG_BUF", "0"))
    nc = bacc.Bacc(
        "TRN2", target_bir_lowering=False, debug=False, num_devices=NCORES
    )

    xT_in = nc.dram_tensor("xT", [FIN, PAD_N], F16, kind="ExternalInput").ap()
    xT_self_in = nc.dram_tensor(
        "xT_self", [FIN, SHARD_N], F16, kind="ExternalInput"
    ).ap()
    w1h_in = nc.dram_tensor("w1h", [FIN, FH], F16, kind="ExternalInput").ap()
    w2h_in = nc.dram_tensor("w2h", [FH, FO], F16, kind="ExternalInput").ap()
    iota_in = nc.dram_tensor("iota", [P, P], F16, kind="ExternalInput").ap()
    ident_in = nc.dram_tensor("ident", [P, P], F16, kind="ExternalInput").ap()
    idx_in = nc.dram_tensor("idx16", [P, SLOTS // 16], I16, kind="ExternalInput").ap()
    col_in = nc.dram_tensor("colv", [P, NB], F32, kind="ExternalInput").ap()
    negcol_in = nc.dram_tensor("negcolv", [P, NB], F32, kind="ExternalInput").ap()
    dinv_g_in = nc.dram_tensor("dinv_g", [P, SHARD_T], F32, kind="ExternalInput").ap()
    dinv_gsq_in = nc.dram_tensor(
        "dinv_gsq", [P, SHARD_T], F32, kind="ExternalInput"
    ).ap()
    if cfg.use_b2:
        b2rep_in = nc.dram_tensor("b2rep", [P, FO], F32, kind="ExternalInput").ap()
    if cfg.use_b1:
        b1r_in = nc.dram_tensor("b1r", [1, FH], F32, kind="ExternalInput").ap()
        sqd_in = nc.dram_tensor("sqd_r", [1, SHARD_N], F32, kind="ExternalInput").ap()

    table1 = nc.dram_tensor("table1", [PAD_N, TROW], F16, kind="Internal").ap()
    table2 = nc.dram_tensor("table2", [PAD_N, TROW], F16, kind="Internal").ap()
    t2c = nc.dram_tensor("t2c", [SHARD_N, FO], F16, kind="Internal").ap()
    t2full = nc.dram_tensor(
        "t2full", [PAD_N, FO], F16, kind="Internal", addr_space="Shared"
    ).ap()
    out = nc.dram_tensor("out", [SHARD_N, FO], F32, kind="ExternalOutput").ap()

    stage = os.environ.get("BASSGNN_STAGE", "full")
    sel_cnt = [0]

    with tile.TileContext(nc) as tc:
        with tc.tile_pool(name="const", bufs=1) as cpool:
            w1_t = cpool.tile([FIN, FH], F16)
            nc.sync.dma_start(out=w1_t[:], in_=w1h_in[:, :])
            w2_t = cpool.tile([FH, FO], F16)
            nc.sync.dma_start(out=w2_t[:], in_=w2h_in[:, :])
            iota_t = cpool.tile([P, P], F16)
            nc.sync.dma_start(out=iota_t[:], in_=iota_in[:, :])
            ident_t = cpool.tile([P, P], F16)
            nc.sync.dma_start(out=ident_t[:], in_=ident_in[:, :])
            col_t = cpool.tile([P, NB], F32)
            nc.sync.dma_start(out=col_t[:], in_=col_in[:, :])
            negcol_t = cpool.tile([P, NB], F32)
            nc.sync.dma_start(out=negcol_t[:], in_=negcol_in[:, :])
            idx_t = cpool.tile([P, SLOTS // 16], I16)
            nc.sync.dma_start(out=idx_t[:], in_=idx_in[:, :])
            dinv_g = cpool.tile([P, SHARD_T], F32)
            nc.sync.dma_start(out=dinv_g[:], in_=dinv_g_in[:, :])
            dinv_gsq = cpool.tile([P, SHARD_T], F32)
            nc.sync.dma_start(out=dinv_gsq[:], in_=dinv_gsq_in[:, :])
            if cfg.use_b2:
                b2_t = cpool.tile([P, FO], F32)
                nc.sync.dma_start(out=b2_t[:], in_=b2rep_in[:, :])
            if cfg.use_b1:
                b1_t = cpool.tile([1, FH], F32)
                nc.sync.dma_start(out=b1_t[:], in_=b1r_in[:, :])
                sqd_t = cpool.tile([1, SHARD_N], F32)
                nc.sync.dma_start(out=sqd_t[:], in_=sqd_in[:, :])
            # persistent self tiles: layer-1 (own-shard table1 rows) and
            # layer-2 (own t2c tiles, filled during layer-1 aggregation)
            self1_t = cpool.tile([P, SHARD_T * FH], F16)
            self2_t = cpool.tile([P, SHARD_T * FO], F16)
            o1all = cpool.tile([P, SHARD_T * FO], F32)

            # ---- Phase B: table1 = (dinv*x) @ W1, full table per core ----
            with (
                tc.tile_pool(name="phb", bufs=3 + BUF) as bpool,
                tc.tile_pool(name="phb_st", bufs=3 + BUF) as stpool,
                tc.tile_pool(name="phb_ps", bufs=4, space="PSUM") as bpsum,
            ):
                GT = 16
                assert NT % GT == 0
                BPB = 512 // FH   # matmul tiles per PSUM bank
                t1_writes = []  # (pos0, pos1, instruction) for B||L1 deps
                for g8 in range(NT // GT):
                    xt8 = bpool.tile([P, GT * P], F16, tag="xt")
                    nc.sync.dma_start(
                        out=xt8[:], in_=xT_in[:, g8 * GT * P:(g8 + 1) * GT * P]
                    )
                    st8 = stpool.tile([P, GT * FH], F16, tag="st")
                    for bk in range(GT // BPB):
                        hp = bpsum.tile([P, BPB * FH], F32, tag="hp")
                        for i in range(BPB):
                            t = bk * BPB + i
                            nc.tensor.matmul(
                                out=hp[:, i * FH:(i + 1) * FH],
                                lhsT=xt8[:, t * P:(t + 1) * P],
                                rhs=w1_t[:], start=True, stop=True,
                            )
                        dst_sl = st8[:, bk * BPB * FH:(bk + 1) * BPB * FH]
                        if bk % 2 == 0:
                            nc.scalar.activation(
                                out=dst_sl, in_=hp[:],
                                func=mybir.ActivationFunctionType.Copy,
                            )
                        else:
                            nc.vector.tensor_copy(out=dst_sl, in_=hp[:])
                    wri = nc.sync.dma_start(
                        out=table1[
                            g8 * GT * P:(g8 + 1) * GT * P, :FH
                        ].rearrange("(m p) f -> p m f", p=P),
                        in_=st8[:].rearrange("p (m f) -> p m f", m=GT),
                    )
                    t1_writes.append(
                        (g8 * GT * P, (g8 + 1) * GT * P, wri.ins)
                    )
                # self tiles for layer 1: own-shard (dinv*x) @ W1, kept in SBUF
                sg = 0
                while sg < SHARD_T:
                    gt_n = min(GT, SHARD_T - sg)
                    xs = bpool.tile([P, gt_n * P], F16, tag="xt")
                    nc.sync.dma_start(
                        out=xs[:], in_=xT_self_in[:, sg * P:(sg + gt_n) * P]
                    )
                    for bk in range(cdiv(gt_n, BPB)):
                        nbk = min(BPB, gt_n - bk * BPB)
                        hp = bpsum.tile([P, nbk * FH], F32, tag="hp")
                        for i in range(nbk):
                            t = bk * BPB + i
                            nc.tensor.matmul(
                                out=hp[:, i * FH:(i + 1) * FH],
                                lhsT=xs[:, t * P:(t + 1) * P],
                                rhs=w1_t[:], start=True, stop=True,
                            )
                        dst_sl = self1_t[
                            :, (sg + bk * BPB) * FH:(sg + bk * BPB + nbk) * FH
                        ]
                        if bk % 2 == 0:
                            nc.scalar.activation(
                                out=dst_sl, in_=hp[:],
                                func=mybir.ActivationFunctionType.Copy,
                            )
                        else:
                            nc.vector.tensor_copy(out=dst_sl, in_=hp[:])
                    sg += gt_n

            OVL = os.environ.get("BG_OVL", "1") == "1"
            if not OVL:
                tc.strict_bb_all_engine_barrier()

            # ---- aggregation over edges ----
            # Selectors are built into "supertiles" of SELK blocks each so
            # the cross-engine WAR semaphore (a separate EventSemaphore
            # instruction costing ~70ns of DVE SEQ) is paid once per SELK
            # blocks instead of per block.  Each builder engine gets its own
            # supertile stream.  Act builds one-hot via
            # relu(1 - |iota - col|) (exact for integer cols); DVE/Pool via
            # tensor_scalar(is_equal).
            SELK = int(os.environ.get("BG_SELK", "16"))
            sel_state = {}

            def build_sel(spool, b, mix, tmp_pool):
                """One-hot selector for block b (slice of a supertile)."""
                c = sel_cnt[0]
                sel_cnt[0] += 1
                e = mix[c % len(mix)]
                st = sel_state.setdefault(e, {"tile": None, "q": SELK})
                if st["q"] >= SELK:
                    sbig = spool.tile([P, SELK * P], F16, tag="s" + e)
                    st["tile"] = sbig
                    st["q"] = 0
                q = st["q"]
                st["q"] += 1
                s_t = st["tile"][:, q * P:(q + 1) * P]
                if e == "a":
                    a_t = tmp_pool.tile([P, P], F16, tag="atmp")
                    nc.scalar.activation(
                        out=a_t[:], in_=iota_t[:],
                        func=mybir.ActivationFunctionType.Abs,
                        bias=negcol_t[:, b:b + 1],
                    )
                    nc.scalar.activation(
                        out=s_t, in_=a_t[:],
                        func=mybir.ActivationFunctionType.Relu,
                        scale=-1.0, bias=1.0,
                    )
                    return s_t
                eng = nc.gpsimd if e == "p" else nc.vector
                eng.tensor_scalar(
                    out=s_t, in0=iota_t[:],
                    scalar1=col_t[:, b:b + 1], scalar2=None,
                    op0=mybir.AluOpType.is_equal,
                )
                return s_t

            def agg_layer(layer):
                FA = FH if layer == 1 else FO
                nchunks = len(cfg.chunks)
                sel_state.clear()
                mix = os.environ.get(
                    "BG_MIX1" if layer == 1 else "BG_MIX2",
                    "d" if layer == 1 else "dddddda",
                )
                SB = int(os.environ.get("BG_SBUF", "6"))
                with (
                    tc.tile_pool(name=f"msg{layer}", bufs=MBUF) as mpool,
                    tc.tile_pool(name=f"s{layer}", bufs=SB) as spool,
                    tc.tile_pool(name=f"work{layer}", bufs=3 + BUF) as wpool,
                    tc.tile_pool(name=f"ps{layer}", bufs=5, space="PSUM") as ppool,
                    tc.tile_pool(name=f"ps{layer}b", bufs=2, space="PSUM") as qpool,
                    tc.tile_pool(name=f"at{layer}", bufs=2) as atpool,
                    tc.tile_pool(name=f"sm{layer}", bufs=1) as smpool,
                ):
                    def issue_gathers(ci):
                        sc0 = cfg.chunk_slot0[ci]
                        j0, j1 = cfg.chunks[ci]
                        cslots = sum(
                            int(cfg.nblk[j, w]) * P
                            for j in range(j0, j1) for w in range(NW)
                        )
                        ckb = cslots // P
                        msg = mpool.tile([P, ckb * FA], F16, tag="msg")
                        for (w, sw0, nw_slots) in cfg.gathers[ci]:
                            bw0 = (sw0 - sc0) // P
                            nbw = nw_slots // P
                            if layer == 1:
                                src_ap = table1[WB[w]:WB[w + 1], :FA]
                            else:
                                src_ap = table2[WB[w]:WB[w + 1], :FA]
                            g_i = raw_gather(
                                nc,
                                out_ap=msg[
                                    :, bw0 * FA:(bw0 + nbw) * FA
                                ].rearrange("p (b f) -> p b f", f=FA),
                                in_ap=src_ap,
                                idxs_ap=idx_t[
                                    :, sw0 // 16: (sw0 + nw_slots) // 16
                                ],
                                num_idxs=nw_slots,
                                elem_size=FA,
                            )
                            if OVL and layer == 1:
                                for (p0, p1, wi) in t1_writes:
                                    if p0 < WB[w + 1] and p1 > WB[w]:
                                        add_dep_helper(
                                            g_i.ins, wi,
                                            reason="gather after t1 write",
                                        )
                        return msg

                    def softmax_batch(t0, t1):
                        # log_softmax + output DMA for tiles [t0, t1)
                        nt = t1 - t0
                        osl = o1all[:, t0 * FO:t1 * FO]
                        o3 = osl.rearrange("p (t f) -> p t f", t=nt)
                        nmx = wpool.tile([P, nt], F32, tag="nmx")
                        nc.vector.tensor_reduce(
                            out=nmx[:], in_=o3,
                            axis=mybir.AxisListType.X,
                            op=mybir.AluOpType.max, negate=True,
                        )
                        nmb = nmx[:].rearrange(
                            "p (t one) -> p t one", one=1
                        ).to_broadcast([P, nt, FO])
                        nc.vector.tensor_tensor(
                            out=o3, in0=o3, in1=nmb, op=mybir.AluOpType.add,
                        )
                        ebig = smpool.tile([P, nt * FO], F32, tag="ebig")
                        nc.scalar.activation(
                            out=ebig[:], in_=osl,
                            func=mybir.ActivationFunctionType.Exp,
                        )
                        ssx = wpool.tile([P, nt], F32, tag="ssx")
                        nc.vector.tensor_reduce(
                            out=ssx[:],
                            in_=ebig[:].rearrange("p (t f) -> p t f", t=nt),
                            axis=mybir.AxisListType.X,
                            op=mybir.AluOpType.add,
                        )
                        lsx = wpool.tile([P, nt], F32, tag="lsx")
                        nc.scalar.activation(
                            out=lsx[:], in_=ssx[:],
                            func=mybir.ActivationFunctionType.Ln,
                        )
                        lsb = lsx[:].rearrange(
                            "p (t one) -> p t one", one=1
                        ).to_broadcast([P, nt, FO])
                        otall = smpool.tile([P, nt * FO], F32, tag="otall")
                        nc.vector.tensor_tensor(
                            out=otall[:].rearrange("p (t f) -> p t f", t=nt),
                            in0=o3, in1=lsb, op=mybir.AluOpType.subtract,
                        )
                        nc.sync.dma_start(
                            out=out[t0 * P:t1 * P, :].rearrange(
                                "(t p) f -> p t f", p=P
                            ),
                            in_=otall[:].rearrange("p (t f) -> p t f", t=nt),
                        )

                    NSM = int(os.environ.get("BG_NSM", "2"))
                    sm_bounds = [
                        cfg.chunks[(len(cfg.chunks) * (i + 1)) // NSM - 1][1]
                        for i in range(NSM)
                    ]
                    sm_done = 0
                    msgs = {}
                    for ci0 in range(min(MBUF - 1, nchunks)):
                        msgs[ci0] = issue_gathers(ci0)
                    for ci, (j0, j1) in enumerate(cfg.chunks):
                        nxt = ci + MBUF - 1
                        if nxt < nchunks:
                            msgs[nxt] = issue_gathers(nxt)
                        msg = msgs.pop(ci)
                        sc0 = cfg.chunk_slot0[ci]
                        if layer == 1:
                            st2big = self2_t[:, j0 * FO:j1 * FO]
                        else:
                            o1big = o1all[:, j0 * FO:j1 * FO]
                        for j in range(j0, j1):
                            blocks = cfg.blocks_of_tile[j]
                            nb = len(blocks)
                            ti = j - j0
                            if layer == 1:
                                gt = ppool.tile([FA, P], F32, tag="gt")
                                noself = os.environ.get("BG_NOSELF") == "1"
                                # self-loop contribution: table1 row d for
                                # dest d = self1_t tile j (identity selector)
                                if not noself:
                                    nc.tensor.matmul(
                                        out=gt[:],
                                        lhsT=self1_t[:, j * FH:(j + 1) * FH],
                                        rhs=ident_t[:],
                                        start=True, stop=False,
                                    )
                                for i, b in enumerate(blocks):
                                    bl = b - sc0 // P
                                    s_t = build_sel(spool, b, mix, atpool)
                                    nc.tensor.matmul(
                                        out=gt[:],
                                        lhsT=msg[:, bl * FA:(bl + 1) * FA],
                                        rhs=s_t[:],
                                        start=(noself and i == 0),
                                        stop=(i == nb - 1 and not cfg.use_b1),
                                    )
                                if cfg.use_b1:
                                    nc.tensor.matmul(
                                        out=gt[:],
                                        lhsT=b1_t[:, :],
                                        rhs=sqd_t[:, j * P:(j + 1) * P],
                                        start=False,
                                        stop=True,
                                    )
                                r1 = wpool.tile([FH, P], F16, tag="r1")
                                nc.scalar.activation(
                                    out=r1[:], in_=gt[:],
                                    func=mybir.ActivationFunctionType.Relu,
                                )
                                h2p = qpool.tile([P, FO], F32, tag="h2")
                                nc.tensor.matmul(
                                    out=h2p[:], lhsT=r1[:], rhs=w2_t[:],
                                    start=True, stop=True,
                                )
                                nc.scalar.activation(
                                    out=st2big[:, ti * FO:(ti + 1) * FO],
                                    in_=h2p[:],
                                    func=mybir.ActivationFunctionType.Copy,
                                    scale=dinv_gsq[:, j:j + 1],
                                )
                            else:
                                # flipped: G[128 dest, FO] += S.T @ msg
                                gt2 = ppool.tile([P, FO], F32, tag="gt2")
                                noself = os.environ.get("BG_NOSELF") == "1"
                                # self-loop: t2full row d for dest d =
                                # own t2c tile j (kept in SBUF)
                                if not noself:
                                    nc.tensor.matmul(
                                        out=gt2[:],
                                        lhsT=ident_t[:],
                                        rhs=self2_t[:, j * FO:(j + 1) * FO],
                                        start=True, stop=False,
                                    )
                                for i, b in enumerate(blocks):
                                    bl = b - sc0 // P
                                    s_t = build_sel(spool, b, mix, atpool)
                                    nc.tensor.matmul(
                                        out=gt2[:],
                                        lhsT=s_t[:],
                                        rhs=msg[:, bl * FA:(bl + 1) * FA],
                                        start=(noself and i == 0),
                                        stop=(i == nb - 1),
                                    )
                                o1 = o1all[:, j * FO:(j + 1) * FO]
                                nc.vector.tensor_scalar(
                                    out=o1, in0=gt2[:],
                                    scalar1=dinv_g[:, j:j + 1], scalar2=None,
                                    op0=mybir.AluOpType.mult,
                                )
                                if cfg.use_b2:
                                    nc.vector.tensor_tensor(
                                        out=o1, in0=o1, in1=b2_t[:],
                                        op=mybir.AluOpType.add,
                                    )
                        if layer == 1:
                            nc.sync.dma_start(
                                out=t2c[
                                    j0 * P:j1 * P, :
                                ].rearrange("(t p) f -> p t f", p=P),
                                in_=st2big[:].rearrange(
                                    "p (t f) -> p t f", t=j1 - j0
                                ),
                            )
                            if layer == 2 and j1 in sm_bounds:
                                softmax_batch(sm_done, j1)
                                sm_done = j1
                            pb = cfg.part_base[ci]
                            prows = NCORES * (j1 - j0) * P
                            if stage == "full":
                                # chunked AllGather overlapped with compute;
                                # pi layout makes the output contiguous
                                nc.gpsimd.collective_compute(
                                    "AllGather",
                                    mybir.AluOpType.bypass,
                                    replica_groups=[list(range(NCORES))],
                                    ins=[t2c[j0 * P:j1 * P, :]],
                                    outs=[t2full[pb:pb + prows, :]],
                                )
                            # expand this part into the 256B-strided table2
                            # (HW gather rows must be 256B-stride aligned)
                            nc.sync.dma_start(
                                out=table2[pb:pb + prows, :FO],
                                in_=t2full[pb:pb + prows, :],
                            )

                    if layer == 2:
                        pass  # softmax batches emitted inline below

            if stage != "b":
                agg_layer(1)

            if stage in ("full", "nocoll"):
                (None if os.environ.get("BG_NOBAR") == "1"
                 else tc.strict_bb_all_engine_barrier())
                agg_layer(2)

    nc.compile()
    return nc


_CACHE = {}
TRACE = False
LAST = None


def kernel(x, edge_index, W1, b1, W2, b2):
    global LAST
    x = np.asarray(x)
    N = x.shape[0]
    cfg, in_maps = prep(x, edge_index, W1, b1, W2, b2)
    key = (
        N, cfg.FIN, cfg.FH, cfg.FO, cfg.SLOTS, cfg.use_b1, cfg.use_b2,
        tuple(cfg.win_bounds),
        tuple(cfg.nblk.reshape(-1).tolist()),
    )
    if key not in _CACHE:
        _CACHE[key] = build_program(cfg)
    nc = _CACHE[key]
    try:
        res = run_bass_kernel_spmd(
            nc, in_maps, core_ids=list(range(NCORES)), trace=TRACE
        )
    except Exception:
        # transient device wedge (NRT_EXEC_UNIT_UNRECOVERABLE) -- retry once
        import time as _time
        _time.sleep(10)
        res = run_bass_kernel_spmd(
            nc, in_maps, core_ids=list(range(NCORES)), trace=TRACE
        )
    LAST = res
    outs = [res.results[k]["out"] for k in range(NCORES)]
    full = np.concatenate(outs, axis=0)[:N]
    return full.astype(np.float32)
